# revision 8
# baseline (speedup 1.0000x reference)
"""Depthwise Conv3D (3x3x3, VALID, stride 1) on 8 Trainium2 NeuronCores — v2.

Strategy: (d,h)-patch stationary matmul. For each channel f and kw tap,
a [126, 84] stationary S maps input patches (pd, ph) in 9x14 onto output
positions (oh, od) in 12x7, folding BOTH kd and kh taps:
    S[pd*14+ph, oh*7+od] = w[pd-od, ph-oh, kw, f]
PSUM accumulates only over the kw=3 taps (vs 9 passes for the Toeplitz-
over-H formulation), with the kw shift expressed as a +kw column offset
into a flat (h-tile, w) moving slab — so each output column costs
3 streamed PE columns instead of 9.

The moving slab per (core, f) is [126, 10*112+4]: partition (pd, ph),
column (t, w) = x[pd, 12*t+ph, w, f]; h-tiles of 12 output rows need
14 input rows (2-row halo, 1.25x input inflation). h rows >= 112 are
zero-padded on host. Everything ships bf16 (rel err ~4e-3 << 2e-2),
including the output, which the host casts back to f32.

Stationaries (derived from the 7KB weight tensor) and bias are loaded
to SBUF once, outside the timing loop, like the baseline's bias.

Sharding: data-parallel over (batch, D-half) -> 8 shards.
"""

import sys

sys.path.insert(0, "/opt/trn_rl_repo")

from contextlib import ExitStack

import numpy as np

B, D, H, W, F = 4, 16, 112, 112, 64
DO, HO, WO = 14, 110, 110
N_CORES = 8
DO_C = 7  # output d-planes per core
DIN_C = 9  # input d-planes per core
HT = 12  # output h rows per tile
NT = 10  # h tiles (covers 120 >= 110 output rows)
PIN = DIN_C * (HT + 2)  # 126 contraction partitions (pd, ph)
POUT = DO_C * HT  # 84 output partitions (oh*7+od)
NCOL = NT * W  # 1120 moving columns (t, w)
NCOLP = NCOL + 4  # padded for +kw offsets
FG = 8  # channels per slab DMA / stage group
CHUNKS = [(0, 4), (448, 4), (896, 2)]  # (col offset, h-tiles) per PSUM bank

_cached = None


def _build(loop_n: int = 1, unroll: bool = False):
    from concourse import bacc, mybir, tile

    nc = bacc.Bacc("TRN2", target_bir_lowering=False, debug=False, num_devices=N_CORES)
    f32 = mybir.dt.float32
    bf16 = mybir.dt.bfloat16

    x_ap = nc.dram_tensor("xp", [F // FG, PIN, FG, NCOLP], bf16, kind="ExternalInput").ap()
    s_ap = nc.dram_tensor("stat", [PIN, F, 3, POUT], bf16, kind="ExternalInput").ap()
    b_ap = nc.dram_tensor("biasbc", [128, F], f32, kind="ExternalInput").ap()
    o_ap = nc.dram_tensor("out", [F // FG, POUT, NT, FG, WO], bf16, kind="ExternalOutput").ap()

    with tile.TileContext(nc) as tc, ExitStack() as ctx:
        const_pool = ctx.enter_context(tc.tile_pool(name="const", bufs=1))
        slab_pool = ctx.enter_context(tc.tile_pool(name="slab", bufs=4))
        stage_pool = ctx.enter_context(tc.tile_pool(name="stage", bufs=2))
        psum_pool = ctx.enter_context(tc.tile_pool(name="psum", bufs=2, space="PSUM"))

        stat_t = const_pool.tile([PIN, F, 3, POUT], bf16, name="stat_t")
        bias_t = const_pool.tile([128, F], f32, name="bias_t")
        nc.sync.dma_start(out=stat_t[:], in_=s_ap[:])
        nc.sync.dma_start(out=bias_t[:], in_=b_ap[:])

        if loop_n > 1 and not unroll:
            ctx.enter_context(tc.For_i(0, loop_n))
        n_rep = loop_n if (loop_n > 1 and unroll) else 1
        NG = F // FG

        slabs = {}

        def load_slab(g):
            if g < n_rep * NG:
                slabs[g] = slab_pool.tile(
                    [PIN, FG, NCOLP], bf16, name="slab", tag="slab"
                )
                nc.gpsimd.dma_start(out=slabs[g], in_=x_ap[g % NG])

        load_slab(0)
        load_slab(1)
        for g in range(n_rep * NG):
            fg = g % NG
            load_slab(g + 2)
            stage = stage_pool.tile([POUT, NT, FG, WO], bf16, name="stage", tag="stage")
            slab = slabs.pop(g)
            for fi in range(FG):
                f = fg * FG + fi
                ps = [
                    psum_pool.tile(
                        [POUT, nt, W],
                        f32,
                        name=f"ps{ci}",
                        tag=f"ps{ci}",
                        bufs=pb,
                    )
                    for (ci, (_, nt)), pb in zip(enumerate(CHUNKS), (3, 3, 2))
                ]
                for kw in range(3):
                    for ci, (c0, nt) in enumerate(CHUNKS):
                        nc.tensor.matmul(
                            ps[ci][:],
                            lhsT=stat_t[:, f, kw, :],
                            rhs=slab[:, fi, c0 + kw : c0 + kw + nt * W],
                            start=(kw == 0),
                            stop=(kw == 2),
                        )
                t0 = 0
                for ci, (c0, nt) in enumerate(CHUNKS):
                    if f % 2 == 0:
                        nc.vector.tensor_scalar_add(
                            stage[:, t0 : t0 + nt, fi, :],
                            ps[ci][:, :, 0:WO],
                            bias_t[0:POUT, f : f + 1],
                        )
                    else:
                        nc.scalar.activation(
                            stage[:, t0 : t0 + nt, fi, :],
                            ps[ci][:, :, 0:WO],
                            mybir.ActivationFunctionType.Identity,
                            bias=bias_t[0:POUT, f : f + 1],
                        )
                    t0 += nt
            # t<9 full; t=9 only oh 0..1 (partitions 0..13) are real rows
            nc.sync.dma_start(out=o_ap[fg][:, 0:9], in_=stage[:, 0:9])
            nc.sync.dma_start(out=o_ap[fg][0:14, 9], in_=stage[0:14, 9])

    nc.compile()
    return nc


def _bf16():
    import ml_dtypes

    return ml_dtypes.bfloat16


def _stationary(w: np.ndarray) -> np.ndarray:
    """w [3,3,3,1,F] -> [PIN, F, 3, POUT] bf16."""
    S = np.zeros((PIN, F, 3, POUT), np.float32)
    for od in range(DO_C):
        for oh in range(HT):
            o = oh * DO_C + od
            for kd in range(3):
                for kh in range(3):
                    p = (od + kd) * (HT + 2) + (oh + kh)
                    S[p, :, :, o] = w[kd, kh, :, 0, :].T
    return S.astype(_bf16())


def _pack_x(xs: np.ndarray) -> np.ndarray:
    """xs [DIN_C, H, W, F] f32 -> [F//FG, PIN, FG, NCOLP] bf16 slab."""
    xpad = np.zeros((DIN_C, NT * HT + 2, W, F), np.float32)
    xpad[:, :H] = xs
    idx = HT * np.arange(NT)[:, None] + np.arange(HT + 2)[None, :]
    xv = xpad[:, idx]  # [pd, t, ph, w, f]
    xv = xv.transpose(0, 2, 4, 1, 3).reshape(PIN, F, NCOL)
    xp = np.zeros((PIN, F, NCOLP), _bf16())
    xp[:, :, :NCOL] = xv.astype(_bf16())
    return np.ascontiguousarray(
        xp.reshape(PIN, F // FG, FG, NCOLP).transpose(1, 0, 2, 3)
    )


def _unpack_out(r: np.ndarray) -> np.ndarray:
    """r [F//FG, POUT, NT, FG, WO] bf16 -> [DO_C, HO, WO, F] f32."""
    r = np.asarray(r, np.float32)
    r = r.transpose(1, 2, 4, 0, 3).reshape(POUT, NT, WO, F)
    r = r.reshape(HT, DO_C, NT, WO, F).transpose(1, 2, 0, 3, 4)
    return np.ascontiguousarray(r.reshape(DO_C, NT * HT, WO, F)[:, :HO])


def _in_maps(x: np.ndarray, w: np.ndarray, b: np.ndarray) -> list:
    x = np.asarray(x, np.float32)
    stat = _stationary(np.asarray(w, np.float32))
    bias_bc = np.tile(np.asarray(b, np.float32)[None, :], (128, 1))

    in_maps = []
    for core in range(N_CORES):
        bb, dh = divmod(core, 2)
        in_maps.append(
            {
                "xp": _pack_x(x[bb, dh * DO_C : dh * DO_C + DIN_C]),
                "stat": stat,
                "biasbc": bias_bc,
            }
        )
    return in_maps


def kernel(x: np.ndarray, w: np.ndarray, b: np.ndarray) -> np.ndarray:
    global _cached
    if _cached is None:
        _cached = _build()
    nc = _cached

    from concourse.bass_utils import run_bass_kernel_spmd

    res = run_bass_kernel_spmd(nc, _in_maps(x, w, b), list(range(N_CORES)))

    out = np.empty((B, DO, HO, WO, F), np.float32)
    for core in range(N_CORES):
        bb, dh = divmod(core, 2)
        out[bb, dh * DO_C : (dh + 1) * DO_C] = _unpack_out(res.results[core]["out"])
    return out
